# revision 19
# baseline (speedup 1.0000x reference)
"""MoE feed-forward (top-2 routing, E=8 experts) on 8 TRN2 NeuronCores.

Strategy: 8-way HIDDEN-dim split with host-side routing/dispatch.
  Every core processes ALL experts' gathered tokens, but only a 1/8
  slice of the hidden dimension (4 of 32 ht tiles of 128). Per-core
  work is exactly sum(c_e)/8 token-equivalents -- perfectly balanced
  regardless of routing skew -- and every core runs the SAME static
  program (per-core variation is only in which weight rows/cols the
  host gathers into the same-shaped input arrays).

  Per expert e and token block b (tokens in [feature, token] layout):
    P1: h[4x128, b] = silu((w1 slice) @ x[b]) * ((w3 slice) @ x[b])
    P2: y_partial[D, b] = (w2 slice) @ h  (contraction over the 512
        local h rows).
  The host scales per-token by the routing weight while scatter-adding
  the 8 per-core partial outputs (f32) into [T, D].

Pipeline layout (from trace analysis of the previous version):
  - All queues are in-order; a DMA whose WAR semaphore isn't ready
    head-of-line-blocks everything behind it. So: out-stores ride
    nc.sync alone; x-preloads ride nc.scalar + nc.gpsimd; weights ride
    nc.gpsimd. xt is 5-deep so a preload issued 2 blocks ahead never
    waits on its WAR semaphore at the queue head.
  - pg/pu PSUM tiles are independently double-buffered (4 banks) and
    ps2 4-deep (4 banks): P1 group k+1 never waits on the silu/mul
    readers of group k. PE gaps >3us also drop the PE to a 2x-slower
    p-state, so each avoided gap pays twice.
  - P1 of block k and P2 of block k-1 interleave on the PE at the
    accumulation-group level; the pipeline runs across expert
    boundaries.
"""
import sys

if "/opt/trn_rl_repo" not in sys.path:
    sys.path.insert(0, "/opt/trn_rl_repo")

import numpy as np
import ml_dtypes

import concourse.bass as bass
import concourse.mybir as mybir
from concourse import bacc
from concourse.tile import TileContext
from concourse.bass_utils import run_bass_kernel_spmd

BF16 = ml_dtypes.bfloat16
P = 128
D = 2048     # model dim
H = 4096     # hidden dim
E = 8        # experts
TOP_K = 2
DO = D // P          # 16 contraction tiles for P1
HL = H // P // E     # 4 local ht tiles per core
DT = D // P          # 16 output-row tiles for P2
B16 = 448            # max token block


def _route(x, router_w):
    """Top-2 expert selection + renormalized weights (float64 host math)."""
    logits = x.astype(np.float64) @ router_w.astype(np.float64).T
    m = logits.max(axis=1, keepdims=True)
    p = np.exp(logits - m)
    p /= p.sum(axis=1, keepdims=True)
    sel = np.argsort(-p, axis=1, kind="stable")[:, :TOP_K]
    rw = np.take_along_axis(p, sel, axis=1)
    rw /= rw.sum(axis=1, keepdims=True)
    return sel, rw.astype(np.float32)


def _seg_blocks(L, maxb, first_small=False, last_small=False):
    """Split a 16-multiple segment into 16-multiple blocks <= maxb.
    first_small carves a 256 head (shrinks the startup DMA wall);
    last_small carves a 96 tail (shrinks the final P2 drain)."""
    if L == 0:
        return []
    head = tail = 0
    if first_small and L > 256:
        head, L = 256, L - 256
    if last_small and L > 256:
        tail, L = 96, L - 96
    sizes = []
    if L:
        n = -(-L // maxb)
        base = (L // n) // 16 * 16
        k = (L - base * n) // 16
        sizes = [base + 16] * k + [base] * (n - k)
    if head:
        sizes = [head] + sizes
    if tail:
        sizes = sizes + [tail]
    return sizes


def _plan(counts16):
    """Block plan: list of (expert, seg_off, tn)."""
    blocks = []
    off16 = 0
    for e in range(E):
        t = 0
        # no first_small head: a small head block makes startup P1 groups
        # SHORTER than the HBM feed rate for their weights (1 MB per group
        # at ~2.8us/MB); natural ~344-token groups run ~4.8us and the ramp
        # stays fed.
        for tn in _seg_blocks(counts16[e], B16, last_small=(e == E - 1)):
            blocks.append((e, off16 + t, tn))
            t += tn
        off16 += counts16[e]
    return blocks, max(off16, 16)


def _build(counts16):
    """Bass program: per-core 1/8-H slice of all experts' GLU MLPs."""
    f32 = mybir.dt.float32
    bf16 = mybir.dt.bfloat16
    Silu = mybir.ActivationFunctionType.Silu

    blocks, S16 = _plan(counts16)

    nc = bacc.Bacc(None, target_bir_lowering=False)
    x16d = nc.dram_tensor("x16d", [P, DO, S16], bf16, kind="ExternalInput")
    w1d16 = nc.dram_tensor("w1d16", [P, E, HL, DO, P], bf16, kind="ExternalInput")
    w3d16 = nc.dram_tensor("w3d16", [P, E, HL, DO, P], bf16, kind="ExternalInput")
    w2d16 = nc.dram_tensor("w2d16", [P, E, 4, 4, HL, P], bf16, kind="ExternalInput")
    o16d = nc.dram_tensor("o16d", [D, S16], bf16, kind="ExternalOutput")
    o16v = o16d.rearrange("(dt p) c -> p dt c", p=P)

    with TileContext(nc) as tc:
        with (
            tc.tile_pool(name="x16p", bufs=5) as x16p,
            tc.tile_pool(name="h16p", bufs=3) as h16p,
            tc.tile_pool(name="w1p16", bufs=8) as w1p16,
            tc.tile_pool(name="w3p16", bufs=8) as w3p16,
            tc.tile_pool(name="w2p16", bufs=8) as w2p16,
            tc.tile_pool(name="stp", bufs=3) as stp,
            tc.tile_pool(name="yp", bufs=6) as yp,
            tc.tile_pool(name="ps13", bufs=2, space="PSUM") as ps13,
            tc.tile_pool(name="ps2", bufs=4, space="PSUM") as ps2,
        ):
            w1t16 = [[None] * HL for _ in range(E)]
            w3t16 = [[None] * HL for _ in range(E)]
            w2t16 = [[None] * 4 for _ in range(E)]

            def load_w13(e, hl, q1=None, q3=None):
                w1t16[e][hl] = w1p16.tile([P, DO, P], bf16, name="w1t16")
                (q1 or nc.gpsimd).dma_start(out=w1t16[e][hl][:], in_=w1d16[:, e, hl])
                w3t16[e][hl] = w3p16.tile([P, DO, P], bf16, name="w3t16")
                (q3 or nc.gpsimd).dma_start(out=w3t16[e][hl][:], in_=w3d16[:, e, hl])

            def load_w2(e, g, q=None):
                w2t16[e][g] = w2p16.tile([P, 4, HL, P], bf16, name="w2t16")
                (q or nc.gpsimd).dma_start(out=w2t16[e][g][:], in_=w2d16[:, e, g])

            def wload_list(e):
                """Closures emitting expert e's weight DMAs, in need order."""
                ops = []
                for hl in range(HL):
                    ops.append(lambda e=e, hl=hl: load_w13(e, hl))
                for g in range(4):
                    ops.append(lambda e=e, g=g: load_w2(e, g))
                return ops

            def load_block(blk, xt):
                """x rides the SAME gpsimd ring as the weight stream, posted
                at block start = ahead of that block's weight closures, so
                the single FIFO serves transfers in exact need order. A
                different ring would starve: SDMA engines drain a whole
                packet per ring, and the SWDGE weight ring wins ~10:1 over
                an HWDGE ring whenever it has backlog (measured: 0.44MB x
                taking 13-18us on sync during weight bursts). Never on
                scalar either: a DMA ahead of a silu head-of-line-blocks
                it. xt is 5 blocks deep so the WAR semaphore is satisfied
                at issue."""
                e, o, tn = blk
                h = DO // 2
                nc.gpsimd.dma_start(out=xt[:, :h, :tn], in_=x16d[:, :h, o : o + tn])
                nc.gpsimd.dma_start(out=xt[:, h:, :tn], in_=x16d[:, h:, o : o + tn])

            def p1_group(blk, xt, ht, hl):
                e, o, tn = blk
                pg = ps13.tile([P, 512], f32, name="pg", tag="pg")[:, :tn]
                pu = ps13.tile([P, 512], f32, name="pu", tag="pu")[:, :tn]
                for dk in range(DO):
                    nc.tensor.matmul(
                        pg, w1t16[e][hl][:, dk], xt[:, dk, :tn],
                        start=(dk == 0), stop=(dk == DO - 1),
                    )
                    nc.tensor.matmul(
                        pu, w3t16[e][hl][:, dk], xt[:, dk, :tn],
                        start=(dk == 0), stop=(dk == DO - 1),
                    )
                st = stp.tile([P, 512], f32, name="st")[:, :tn]
                nc.scalar.activation(st, pg, Silu)
                nc.vector.tensor_mul(ht[:, hl, :tn], st, pu)

            def p2_group(blk, ht, dt, drain=False):
                e, o, tn = blk
                # final drain: P1 is done, so its 4 PSUM banks are free --
                # alternate pools for an 8-deep rotation so the PE never
                # waits on the copy/store chain.
                if drain and dt % 2 == 1:
                    py = ps13.tile([P, 512], f32, name="py2", tag="pg" if dt % 4 == 1 else "pu")[:, :tn]
                else:
                    py = ps2.tile([P, 512], f32, name="py")[:, :tn]
                for hl in range(HL):
                    nc.tensor.matmul(
                        py, w2t16[e][dt // 4][:, dt % 4, hl], ht[:, hl, :tn],
                        start=(hl == 0), stop=(hl == HL - 1),
                    )
                ysb = yp.tile([P, 512], bf16, name="ysb")[:, :tn]
                # final drain: alternate the PSUM->SBUF copy between vector
                # and scalar so the 16-copy chain after the last matmul
                # doesn't serialize on one engine.
                if drain and dt % 2 == 1:
                    nc.scalar.activation(ysb, py, mybir.ActivationFunctionType.Copy)
                else:
                    nc.vector.tensor_copy(ysb, py)
                # final expert: split out-DMAs with the (by now idle) weight
                # queue so the drain tail isn't serialized on one queue.
                # Not scalar: the drain's odd-dt copies ride it.
                if e == E - 1:
                    q = (nc.sync, nc.gpsimd)[dt % 2]
                else:
                    q = nc.sync
                q.dma_start(out=o16v[:, dt, o : o + tn], in_=ysb)

            # ---- emission: startup spreads first loads over 3 queues ----
            bq = []  # xt tiles loaded, awaiting compute

            def preload(j):
                blk = blocks[j]
                xt = x16p.tile([P, DO, B16], bf16, name="xt16")
                load_block(blk, xt)
                bq.append((blk, xt))

            # Startup: the first MM needs only x0[dk 0-3] + w13[0][0], so
            # split x block 0 into quarters across all three queues (the
            # dk-0..3 quarter first on gpsimd) with w13[0][0] right behind
            # on sync/scalar. Measured: first MM fires ~10us in vs ~15us
            # with half-granularity loads.
            blk0 = blocks[0]
            _, o0, tn0 = blk0
            xt0 = x16p.tile([P, DO, B16], bf16, name="xt16")
            qs = DO // 4
            for qi, q in enumerate((nc.gpsimd, nc.gpsimd, nc.sync, nc.scalar)):
                q.dma_start(
                    out=xt0[:, qi * qs : (qi + 1) * qs, :tn0],
                    in_=x16d[:, qi * qs : (qi + 1) * qs, o0 : o0 + tn0],
                )
            bq.append((blk0, xt0))
            load_w13(0, 0, q1=nc.sync, q3=nc.scalar)
            load_w13(0, 1)
            load_w13(0, 2)
            load_w13(0, 3)
            preload(1)
            # expert-0 w2 is only needed when block 1 starts (P2 of block 0
            # interleaves with P1 of block 1); defer it into the throttled
            # stream so the startup window stays under the HBM ceiling.
            pending = [lambda g=g: load_w2(0, g) for g in range(4)]
            pending += wload_list(1)
            next_loaded = 1

            prev = None  # (blk, ht) awaiting its P2
            for k, blk in enumerate(blocks):
                e, o, tn = blk
                if k + 2 < len(blocks):
                    preload(k + 2)
                while next_loaded < min(e + 2, E):
                    pending += wload_list(next_loaded)
                    next_loaded += 1
                _, xt = bq.pop(0)
                ht = h16p.tile([P, HL, B16], bf16, name="ht16")
                for hl in range(HL):
                    p1_group(blk, xt, ht, hl)
                    if prev is not None:
                        for dt in range(4 * hl, 4 * hl + 4):
                            p2_group(*prev, dt)
                    # throttle the weight stream: HBM is ~358 GB/s shared
                    # with x-preloads and out-stores, and an over-eager
                    # prefetch starves the x stream during the short
                    # startup blocks (a w13 closure is 1 MB). 1 closure
                    # per P1 group = 4/block vs 8 closures per expert per
                    # >=1 block: stays >=1 expert ahead at ~1/4 the burst
                    # bandwidth.
                    for f in pending[:1]:
                        f()
                    pending = pending[1:]
                prev = (blk, ht)
            for f in pending:
                f()
            for dt in range(DT):
                p2_group(*prev, dt, drain=True)

    nc.compile()
    return nc, blocks, S16


_cache = {}


def _get_program(counts16):
    key = tuple(counts16)
    if key not in _cache:
        _cache[key] = _build(counts16)
    return _cache[key]


def _prep_weights(w1, w3, w2):
    """Per-core weight arrays in device layouts (see _build docstring)."""
    per_core = []
    w1_16 = w1.astype(BF16)
    w3_16 = w3.astype(BF16)
    w2_16 = w2.astype(BF16)
    for i in range(E):
        r0 = 512 * i
        W116 = np.empty((P, E, HL, DO, P), BF16)
        W316 = np.empty((P, E, HL, DO, P), BF16)
        W216 = np.empty((P, E, 4, 4, HL, P), BF16)
        for e in range(E):
            W116[:, e] = w1_16[e, r0 : r0 + 512].reshape(HL, P, DO, P).transpose(3, 0, 2, 1)
            W316[:, e] = w3_16[e, r0 : r0 + 512].reshape(HL, P, DO, P).transpose(3, 0, 2, 1)
            W216[:, e] = (
                w2_16[e, :, r0 : r0 + 512].reshape(4, 4, P, HL, P).transpose(4, 0, 1, 3, 2)
            )
        per_core.append(dict(w1d16=W116, w3d16=W316, w2d16=W216))
    return per_core


_wcache = {"key": None, "val": None}


def kernel(x, router_w, w1, w3, w2, _trace=False):
    T = x.shape[0]
    x = np.asarray(x, np.float32)
    router_w = np.asarray(router_w, np.float32)
    w1 = np.asarray(w1, np.float32)
    w3 = np.asarray(w3, np.float32)
    w2 = np.asarray(w2, np.float32)
    assert x.shape[1] == D and router_w.shape == (E, D)
    assert w1.shape == w3.shape == (E, H, D) and w2.shape == (E, D, H)

    sel, rw = _route(x, router_w)

    # per-expert token lists
    toks, cws = [], []
    for e in range(E):
        mask = sel == e
        tok = np.nonzero(mask.any(axis=1))[0]
        cw = np.where(mask[tok, 0], rw[tok, 0], rw[tok, 1])
        toks.append(tok)
        cws.append(cw)

    counts16 = [-(-len(t) // 16) * 16 if len(t) else 0 for t in toks]
    nc, blocks, S16 = _get_program(counts16)

    # ---- host-side gathers into device layouts ----
    xg16 = np.zeros((S16, D), np.float32)
    o16 = 0
    spans = []  # (e, off16, n16)
    for e in range(E):
        t16 = toks[e]
        xg16[o16 : o16 + len(t16)] = x[t16]
        spans.append((e, o16, len(t16)))
        o16 += counts16[e]

    x16d = np.ascontiguousarray(
        xg16.T.reshape(DO, P, S16).transpose(1, 0, 2).astype(BF16)
    )

    wkey = (x.ctypes.data, w1.ctypes.data, w2.ctypes.data, w3.ctypes.data)
    if _wcache["key"] != wkey:
        _wcache["key"] = wkey
        _wcache["val"] = _prep_weights(w1, w3, w2)
    wmaps = _wcache["val"]

    in_maps = [dict(x16d=x16d, **wmaps[i]) for i in range(E)]
    res = run_bass_kernel_spmd(nc, in_maps, core_ids=list(range(E)), trace=_trace)

    O16 = res.results[0]["o16d"].astype(np.float32)
    for i in range(1, E):
        O16 += res.results[i]["o16d"].astype(np.float32)

    out = np.zeros((T, D), np.float32)
    for e, p16, n16 in spans:
        if n16:
            out[toks[e]] += cws[e][:, None] * O16[:, p16 : p16 + n16].T
    if _trace:
        kernel.last_exec_time_ns = res.exec_time_ns
        kernel.last_results = res
    return out
